# revision 6
# baseline (speedup 1.0000x reference)
"""Content-addressed cache-select kernel for Trainium2 (8 NeuronCores, SPMD).

Problem: out = cached_outputs[idx] where idx is the first row of
`fingerprints` (6x4) exactly equal to the first 4 floats of `x`, else 0.

Strategy (row-parallel over 8 cores):
  - Each core receives its 2048-row shard of all 6 cached slabs plus a
    small staged "meta" vector (fingerprints, the replicated probe tiled
    x6, and index weights) packed on the host.
  - On device: one DMA stages meta into SBUF; the vector engine reduces
    the fingerprint comparison to m = 8 - first_match in 4 small int32
    ops (bitwise equality == float equality for these inputs; first
    match wins, no match -> m=0); the SP and ACT engines each load m,
    finish idx = (8-m)&7 with two register ALU ops, and issue
    dynamic-offset DRAM->DRAM DMAs copying the selected 32MB slab shard
    to the output.
  - The copy (DRAM->DRAM, 64KB descriptors) saturates the ~660GB/s
    per-core HBM read+write ceiling. SDMA engine 15 intermittently runs
    ~18% slow (observed in ~half of profiled runs), so the descriptor
    schedule gives it half-load: part A (contiguous, SP queue) spreads
    17 descriptors to each of the 16 engines; part B (4-row-block
    interleaved AP, ACT queue) gives engines 0-14 another 16. Makespan
    is ~33 descriptors/engine whether or not engine 15 is slow.
"""
import numpy as np

import concourse.bass as bass
import concourse.mybir as mybir
from concourse.bass_utils import run_bass_kernel_spmd

N_CASES = 6
ROWS, COLS = 16384, 4096
N_CORES = 8
RS = ROWS // N_CORES  # rows per core

# Copy split between the two HWDGE queues (SP and Activation issue one DMA
# each). The shapes pin the SDMA engine grouping: part A ([16, 17, 16384]
# after lowering) gives every engine 17 64KB descriptors; part B
# ([15, 16, 16384]) gives engines 0-14 another 16. SDMA engine 15
# intermittently runs ~18% slow (observed in half of profiled runs), so it
# gets half-load; makespan is ~33 descriptors/engine either way.
ROWS_A = 1088  # 16 groups x 68 rows -> 17 x 64KB descriptors per engine
ROWS_B = RS - ROWS_A  # 960 = 15 groups x 64 rows -> 16 x 64KB per engine


def build(rows_a=None, rows_b=None, groups_a=16, groups_b=15):
    rows_a = rows_a or ROWS_A
    rows_b = rows_b if rows_b is not None else ROWS_B
    assert rows_a % groups_a == 0 and rows_b % groups_b == 0
    nc = bass.Bass()
    f32 = mybir.dt.float32
    i32 = mybir.dt.int32

    meta = nc.dram_tensor("meta", [1, 64], i32, kind="ExternalInput")
    cached = nc.dram_tensor("cached", [N_CASES, RS, COLS], f32, kind="ExternalInput")
    out = nc.dram_tensor("out", [RS, COLS], f32, kind="ExternalOutput")

    with (
        nc.sbuf_tensor("stage", [1, 128], i32) as stage,
        nc.Block(no_gpsimd_drain=True) as block,
        nc.semaphore("ssem") as ssem,
        nc.semaphore("vsem") as vsem,
        nc.semaphore("bsem") as bsem,
        nc.semaphore("asem") as asem,
    ):

        @block.sync
        def _(sync):
            sync.dma_start(stage[0:1, 0:64], meta[0:1, 0:64]).then_inc(ssem, 16)

        @block.vector
        def _(vector):
            vector.wait_ge(ssem, 16)
            st = stage
            step = [0]

            def chain(inst):
                step[0] += 1
                inst.then_inc(vsem, 1)
                vector.wait_ge(vsem, step[0])

            # eq[64:88] = (fps == probe_tiled) as int32 0/1 (bitwise equality)
            chain(
                vector.tensor_tensor(
                    st[0:1, 64:88],
                    st[0:1, 0:24],
                    st[0:1, 24:48],
                    mybir.AluOpType.is_equal,
                )
            )
            # all4[88:94] = min over each fingerprint's 4 equality bits
            eq_v = st[0:1, 64:88].rearrange("p (a b) -> p a b", a=6)
            chain(
                vector.tensor_reduce(
                    st[0:1, 88:94], eq_v, mybir.AxisListType.X, mybir.AluOpType.min
                )
            )
            # score[94:100] = all4 * [8,7,6,5,4,3] (weights staged at [48:54])
            chain(
                vector.tensor_tensor(
                    st[0:1, 94:100],
                    st[0:1, 88:94],
                    st[0:1, 48:54],
                    mybir.AluOpType.mult,
                )
            )
            # m[100:101] = max(score) = 8 - first_match (0 if no match).
            # The issuing engines turn m into the index with register ALU
            # ops, which are cheaper than two more DVE ops here.
            chain(
                vector.tensor_reduce(
                    st[0:1, 100:101],
                    st[0:1, 94:100],
                    mybir.AxisListType.X,
                    mybir.AluOpType.max,
                )
            )

        def interleaved(ap, groups, f):
            # [r, COLS] region traversed as [groups, m, f*COLS]: 4-row (64KB)
            # blocks are dealt round-robin to `groups` outer slots, and the
            # strided outer dim survives AP optimization, pinning the SDMA
            # engine grouping to engines 0..groups-1. Same pattern on both
            # sides of the DMA keeps the element mapping the identity.
            if len(ap.shape) == 3:  # dynamic [1, r, COLS] slice of cached
                return ap.rearrange("q (m x f) c -> (q x) m (f c)", x=groups, f=f)
            return ap.rearrange("(m x f) c -> x m (f c)", x=groups, f=f)

        def load_idx(eng, name):
            # idx = (8 - m) & 7: first-match index, no-match m=0 -> 8&7 = 0.
            with eng.register(name) as r:
                eng.reg_load(r, stage[0:1, 100:101])
                eng.reg_alu(r, 8, r, mybir.AluOpType.subtract)
                eng.reg_alu(r, r, 7, mybir.AluOpType.bitwise_and)
                return eng.snap(r, min_val=0, max_val=N_CASES - 1)

        @block.sync
        def _(sync):
            sync.wait_ge(vsem, 4)
            idx = load_idx(sync, "idxr")
            rows = slice(0, rows_a)
            sync.dma_start(
                out[rows, :], cached[bass.ds(idx, 1), rows, :]
            ).then_inc(bsem, 16)
            sync.wait_ge(bsem, 16)

        @block.scalar
        def _(scalar):
            if rows_b == 0:
                return
            scalar.wait_ge(vsem, 4)
            idx2 = load_idx(scalar, "idxa")
            rows = slice(rows_a, RS)
            f = 4 if COLS == 4096 else 1
            scalar.dma_start(
                interleaved(out[rows, :], groups_b, f),
                interleaved(cached[bass.ds(idx2, 1), rows, :], groups_b, f),
            ).then_inc(asem, 16)
            scalar.wait_ge(asem, 16)

    return nc


def build_static(rows_a=None, rows_b=None, groups_a=16, groups_b=15, idx_const=3):
    """Probe variant: same copy structure as build() but NO meta DMA / select —
    the slab index is hardcoded. Isolates the select-path cost and the
    framework pre/postamble. NOT a valid deliverable (cheats the select)."""
    rows_a = rows_a or ROWS_A
    rows_b = rows_b if rows_b is not None else ROWS_B
    nc = bass.Bass()
    f32 = mybir.dt.float32
    i32 = mybir.dt.int32

    nc.dram_tensor("meta", [2, 32], i32, kind="ExternalInput")  # unused
    cached = nc.dram_tensor("cached", [N_CASES, RS, COLS], f32, kind="ExternalInput")
    out = nc.dram_tensor("out", [RS, COLS], f32, kind="ExternalOutput")

    with (
        nc.Block(no_gpsimd_drain=True) as block,
        nc.semaphore("bsem") as bsem,
        nc.semaphore("asem") as asem,
    ):

        def interleaved(ap, groups, f):
            if len(ap.shape) == 3:
                return ap.rearrange("q (m x f) c -> (q x) m (f c)", x=groups, f=f)
            return ap.rearrange("(m x f) c -> x m (f c)", x=groups, f=f)

        @block.sync
        def _(sync):
            rows = slice(0, rows_a)
            sync.dma_start(out[rows, :], cached[idx_const, rows, :]).then_inc(
                bsem, 16
            )
            sync.wait_ge(bsem, 16)

        @block.scalar
        def _(scalar):
            if rows_b == 0:
                return
            rows = slice(rows_a, RS)
            f = 4 if COLS == 4096 else 1
            scalar.dma_start(
                interleaved(out[rows, :], groups_b, f),
                interleaved(cached[idx_const, rows, :], groups_b, f),
            ).then_inc(asem, 16)
            scalar.wait_ge(asem, 16)

    return nc


def build_v2(rows_a=None, rows_b=None, groups_a=16, groups_b=15):
    """Lean-select variant: meta staged as [2,32] (2 descriptors, no 16-way
    spray), 4 DVE ops ending in idx directly (max of hit[i]*i, no-match -> 0),
    no snap/mask on the issuing engines."""
    rows_a = rows_a or ROWS_A
    rows_b = rows_b if rows_b is not None else ROWS_B
    nc = bass.Bass()
    f32 = mybir.dt.float32
    i32 = mybir.dt.int32

    meta = nc.dram_tensor("meta", [2, 32], i32, kind="ExternalInput")
    cached = nc.dram_tensor("cached", [N_CASES, RS, COLS], f32, kind="ExternalInput")
    out = nc.dram_tensor("out", [RS, COLS], f32, kind="ExternalOutput")

    with (
        nc.sbuf_tensor("stage", [1, 128], i32) as stage,
        nc.Block(no_gpsimd_drain=True) as block,
        nc.semaphore("ssem") as ssem,
        nc.semaphore("vsem") as vsem,
        nc.semaphore("bsem") as bsem,
        nc.semaphore("asem") as asem,
    ):

        @block.sync
        def _(sync):
            sync.dma_start(stage[0:1, 0:64], meta[0:2, 0:32]).then_inc(ssem, 16)

        @block.vector
        def _(vector):
            vector.wait_ge(ssem, 16)
            st = stage
            step = [0]

            def chain(inst):
                step[0] += 1
                inst.then_inc(vsem, 1)
                vector.wait_ge(vsem, step[0])

            # eq[p0 32:56] = (fps == probe_tiled): p0[0:24] vs p1[0:24]
            chain(
                vector.tensor_tensor(
                    st[0:1, 64:88],
                    st[0:1, 0:24],
                    st[0:1, 24:48],
                    mybir.AluOpType.is_equal,
                )
            )
            # hit[p0 56:62] = min over each fingerprint's 4 equality bits
            eq_v = st[0:1, 64:88].rearrange("p (a b) -> p a b", a=6)
            chain(
                vector.tensor_reduce(
                    st[0:1, 88:94], eq_v, mybir.AxisListType.X, mybir.AluOpType.min
                )
            )
            # score[p0 62:68] = hit * [0,1,2,3,4,5] (weights staged at p0 24:30)
            chain(
                vector.tensor_tensor(
                    st[0:1, 94:100],
                    st[0:1, 88:94],
                    st[0:1, 48:54],
                    mybir.AluOpType.mult,
                )
            )
            # idx[p0 68] = max(score): the matching case index, 0 if no match.
            chain(
                vector.tensor_reduce(
                    st[0:1, 100:101],
                    st[0:1, 94:100],
                    mybir.AxisListType.X,
                    mybir.AluOpType.max,
                )
            )

        def interleaved(ap, groups, f):
            if len(ap.shape) == 3:
                return ap.rearrange("q (m x f) c -> (q x) m (f c)", x=groups, f=f)
            return ap.rearrange("(m x f) c -> x m (f c)", x=groups, f=f)

        def load_idx(eng, name):
            with eng.register(name) as r:
                eng.reg_load(r, stage[0:1, 100:101])
                return eng.snap(r, donate=True, min_val=0, max_val=N_CASES - 1)

        @block.sync
        def _(sync):
            sync.wait_ge(vsem, 4)
            idx = load_idx(sync, "idxr")
            rows = slice(0, rows_a)
            sync.dma_start(
                out[rows, :], cached[bass.ds(idx, 1), rows, :]
            ).then_inc(bsem, 16)
            sync.wait_ge(bsem, 16)

        @block.scalar
        def _(scalar):
            if rows_b == 0:
                return
            scalar.wait_ge(vsem, 4)
            idx2 = load_idx(scalar, "idxa")
            rows = slice(rows_a, RS)
            f = 4 if COLS == 4096 else 1
            scalar.dma_start(
                interleaved(out[rows, :], groups_b, f),
                interleaved(cached[bass.ds(idx2, 1), rows, :], groups_b, f),
            ).then_inc(asem, 16)
            scalar.wait_ge(asem, 16)

    return nc


def make_meta_v2(probe, fps):
    flat = np.zeros(64, dtype=np.int32)
    flat[0:24] = fps.reshape(-1).view(np.int32)
    flat[24:48] = np.tile(probe.reshape(-1), 6).view(np.int32)
    flat[48:54] = np.arange(6, dtype=np.int32)
    return flat.reshape(2, 32)


def build_v3(rows_a=None, rows_b=None, groups_b=15):
    """v2's lean select, but both copy parts issued from the SP engine on the
    single qSPDynamicHW ring (part B still engine-15-hedged via the
    15-group interleave). Tests whether NRT's dma_rearm postamble scales
    with the number of HWDGE rings used."""
    rows_a = rows_a or ROWS_A
    rows_b = rows_b if rows_b is not None else ROWS_B
    nc = bass.Bass()
    f32 = mybir.dt.float32
    i32 = mybir.dt.int32

    meta = nc.dram_tensor("meta", [2, 32], i32, kind="ExternalInput")
    cached = nc.dram_tensor("cached", [N_CASES, RS, COLS], f32, kind="ExternalInput")
    out = nc.dram_tensor("out", [RS, COLS], f32, kind="ExternalOutput")

    with (
        nc.sbuf_tensor("stage", [1, 128], i32) as stage,
        nc.Block(no_gpsimd_drain=True) as block,
        nc.semaphore("ssem") as ssem,
        nc.semaphore("vsem") as vsem,
        nc.semaphore("bsem") as bsem,
    ):

        @block.sync
        def _(sync):
            sync.dma_start(stage[0:1, 0:64], meta[0:2, 0:32]).then_inc(ssem, 16)

        @block.vector
        def _(vector):
            vector.wait_ge(ssem, 16)
            st = stage
            step = [0]

            def chain(inst):
                step[0] += 1
                inst.then_inc(vsem, 1)
                vector.wait_ge(vsem, step[0])

            chain(
                vector.tensor_tensor(
                    st[0:1, 64:88],
                    st[0:1, 0:24],
                    st[0:1, 24:48],
                    mybir.AluOpType.is_equal,
                )
            )
            eq_v = st[0:1, 64:88].rearrange("p (a b) -> p a b", a=6)
            chain(
                vector.tensor_reduce(
                    st[0:1, 88:94], eq_v, mybir.AxisListType.X, mybir.AluOpType.min
                )
            )
            chain(
                vector.tensor_tensor(
                    st[0:1, 94:100],
                    st[0:1, 88:94],
                    st[0:1, 48:54],
                    mybir.AluOpType.mult,
                )
            )
            chain(
                vector.tensor_reduce(
                    st[0:1, 100:101],
                    st[0:1, 94:100],
                    mybir.AxisListType.X,
                    mybir.AluOpType.max,
                )
            )

        def interleaved(ap, groups, f):
            if len(ap.shape) == 3:
                return ap.rearrange("q (m x f) c -> (q x) m (f c)", x=groups, f=f)
            return ap.rearrange("(m x f) c -> x m (f c)", x=groups, f=f)

        @block.sync
        def _(sync):
            sync.wait_ge(vsem, 4)
            with sync.register("idxr") as r:
                sync.reg_load(r, stage[0:1, 100:101])
                idx = sync.snap(r, donate=True, min_val=0, max_val=N_CASES - 1)
            rows = slice(0, rows_a)
            sync.dma_start(
                out[rows, :], cached[bass.ds(idx, 1), rows, :]
            ).then_inc(bsem, 16)
            rows = slice(rows_a, RS)
            f = 4 if COLS == 4096 else 1
            sync.dma_start(
                interleaved(out[rows, :], groups_b, f),
                interleaved(cached[bass.ds(idx, 1), rows, :], groups_b, f),
            ).then_inc(bsem, 16)
            sync.wait_ge(bsem, 32)

    return nc


def build_v4(rows_a=None, rows_b=None, groups_b=15):
    """v2's lean select without the nc.Block wrapper: instructions are
    emitted straight into the main body, so there are no per-engine block
    branches, no exit drains, and no bass exit barrier (NRT's postamble
    barrier already serializes program end)."""
    rows_a = rows_a or ROWS_A
    rows_b = rows_b if rows_b is not None else ROWS_B
    nc = bass.Bass()
    f32 = mybir.dt.float32
    i32 = mybir.dt.int32

    meta = nc.dram_tensor("meta", [2, 32], i32, kind="ExternalInput")
    cached = nc.dram_tensor("cached", [N_CASES, RS, COLS], f32, kind="ExternalInput")
    out = nc.dram_tensor("out", [RS, COLS], f32, kind="ExternalOutput")

    stage_cm = nc.sbuf_tensor("stage", [1, 128], i32)
    stage = stage_cm.__enter__()
    ssem_cm = nc.semaphore("ssem")
    ssem = ssem_cm.__enter__()
    vsem_cm = nc.semaphore("vsem")
    vsem = vsem_cm.__enter__()
    bsem_cm = nc.semaphore("bsem")
    bsem = bsem_cm.__enter__()
    asem_cm = nc.semaphore("asem")
    asem = asem_cm.__enter__()

    sync, vector, scalar = nc.sync, nc.vector, nc.scalar

    sync.dma_start(stage[0:1, 0:64], meta[0:2, 0:32]).then_inc(ssem, 16)

    vector.wait_ge(ssem, 16)
    st = stage
    step = [0]

    def chain(inst):
        step[0] += 1
        inst.then_inc(vsem, 1)
        vector.wait_ge(vsem, step[0])

    chain(
        vector.tensor_tensor(
            st[0:1, 64:88], st[0:1, 0:24], st[0:1, 24:48], mybir.AluOpType.is_equal
        )
    )
    eq_v = st[0:1, 64:88].rearrange("p (a b) -> p a b", a=6)
    chain(
        vector.tensor_reduce(
            st[0:1, 88:94], eq_v, mybir.AxisListType.X, mybir.AluOpType.min
        )
    )
    chain(
        vector.tensor_tensor(
            st[0:1, 94:100], st[0:1, 88:94], st[0:1, 48:54], mybir.AluOpType.mult
        )
    )
    chain(
        vector.tensor_reduce(
            st[0:1, 100:101], st[0:1, 94:100], mybir.AxisListType.X,
            mybir.AluOpType.max,
        )
    )

    def interleaved(ap, groups, f):
        if len(ap.shape) == 3:
            return ap.rearrange("q (m x f) c -> (q x) m (f c)", x=groups, f=f)
        return ap.rearrange("(m x f) c -> x m (f c)", x=groups, f=f)

    def load_idx(eng, name):
        with eng.register(name) as r:
            eng.reg_load(r, stage[0:1, 100:101])
            return eng.snap(r, donate=True, min_val=0, max_val=N_CASES - 1)

    sync.wait_ge(vsem, 4)
    idx = load_idx(sync, "idxr")
    rows = slice(0, rows_a)
    sync.dma_start(out[rows, :], cached[bass.ds(idx, 1), rows, :]).then_inc(bsem, 16)

    scalar.wait_ge(vsem, 4)
    idx2 = load_idx(scalar, "idxa")
    rows = slice(rows_a, RS)
    f = 4 if COLS == 4096 else 1
    scalar.dma_start(
        interleaved(out[rows, :], groups_b, f),
        interleaved(cached[bass.ds(idx2, 1), rows, :], groups_b, f),
    ).then_inc(asem, 16)

    sync.wait_ge(bsem, 16)
    scalar.wait_ge(asem, 16)

    return nc


VARIANTS = {
    "base": (None, None),  # filled below
    "static": (build_static, "make_meta_v2"),
    "v2": (build_v2, "make_meta_v2"),
    "v3": (build_v3, "make_meta_v2"),
    "v4": (build_v4, "make_meta_v2"),
}


def build_variant(name):
    if name == "base":
        return build()
    return VARIANTS[name][0]()


def make_meta_variant(name, probe, fps):
    if name == "base":
        return make_meta(probe, fps)
    return make_meta_v2(probe, fps)


def make_meta(probe, fps):
    buf = np.zeros((1, 64), dtype=np.int32)
    buf[0, 0:24] = fps.reshape(-1).view(np.int32)
    buf[0, 24:48] = np.tile(probe.reshape(-1), 6).view(np.int32)
    buf[0, 48:54] = np.array([8, 7, 6, 5, 4, 3], dtype=np.int32)
    return buf


def run(inputs, trace=False, **spmd_kwargs):
    x = np.asarray(inputs["x"], dtype=np.float32)
    fingerprints = np.asarray(inputs["fingerprints"], dtype=np.float32)
    cached_outputs = np.asarray(inputs["cached_outputs"], dtype=np.float32)

    nc = build()
    meta = make_meta(x.reshape(-1)[:4], fingerprints)
    in_maps = []
    for c in range(N_CORES):
        shard = np.ascontiguousarray(cached_outputs[:, c * RS : (c + 1) * RS, :])
        in_maps.append({"meta": meta, "cached": shard})

    res = run_bass_kernel_spmd(
        nc, in_maps, list(range(N_CORES)), trace=trace, **spmd_kwargs
    )
    out = np.concatenate([res.results[c]["out"] for c in range(N_CORES)], axis=0)
    return out.astype(np.float32), res


def kernel(**inputs) -> np.ndarray:
    out, _ = run(inputs, trace=False)
    return out



# revision 7
# speedup vs baseline: 1.0477x; 1.0477x over previous
"""Content-addressed cache-select kernel for Trainium2 (8 NeuronCores, SPMD).

Problem: out = cached_outputs[idx] where idx is the first row of
`fingerprints` (6x4) exactly equal to the first 4 floats of `x`, else 0.

Strategy (row-parallel over 8 cores):
  - Each core receives its 2048-row shard of all 6 cached slabs plus a
    small staged "meta" vector (fingerprints, the replicated probe tiled
    x6, and index weights) packed on the host.
  - On device: one DMA stages meta into SBUF; the vector engine reduces
    the fingerprint comparison to m = 8 - first_match in 4 small int32
    ops (bitwise equality == float equality for these inputs; first
    match wins, no match -> m=0); the SP and ACT engines each load m,
    finish idx = (8-m)&7 with two register ALU ops, and issue
    dynamic-offset DRAM->DRAM DMAs copying the selected 32MB slab shard
    to the output.
  - The copy (DRAM->DRAM, 64KB descriptors) saturates the ~660GB/s
    per-core HBM read+write ceiling. SDMA engine 15 intermittently runs
    ~18% slow (observed in ~half of profiled runs), so the descriptor
    schedule gives it half-load: part A (contiguous, SP queue) spreads
    17 descriptors to each of the 16 engines; part B (4-row-block
    interleaved AP, ACT queue) gives engines 0-14 another 16. Makespan
    is ~33 descriptors/engine whether or not engine 15 is slow.
"""
import numpy as np

import concourse.bass as bass
import concourse.mybir as mybir
from concourse.bass_utils import run_bass_kernel_spmd

N_CASES = 6
ROWS, COLS = 16384, 4096
N_CORES = 8
RS = ROWS // N_CORES  # rows per core

# Copy split between the two HWDGE queues (SP and Activation issue one DMA
# each). The shapes pin the SDMA engine grouping: part A ([16, 17, 16384]
# after lowering) gives every engine 17 64KB descriptors; part B
# ([15, 16, 16384]) gives engines 0-14 another 16. SDMA engine 15
# intermittently runs ~18% slow (observed in half of profiled runs), so it
# gets half-load; makespan is ~33 descriptors/engine either way.
ROWS_A = 1088  # 16 groups x 68 rows -> 17 x 64KB descriptors per engine
ROWS_B = RS - ROWS_A  # 960 = 15 groups x 64 rows -> 16 x 64KB per engine


def build(rows_a=None, rows_b=None, groups_a=16, groups_b=15):
    rows_a = rows_a or ROWS_A
    rows_b = rows_b if rows_b is not None else ROWS_B
    assert rows_a % groups_a == 0 and rows_b % groups_b == 0
    nc = bass.Bass()
    f32 = mybir.dt.float32
    i32 = mybir.dt.int32

    meta = nc.dram_tensor("meta", [1, 64], i32, kind="ExternalInput")
    cached = nc.dram_tensor("cached", [N_CASES, RS, COLS], f32, kind="ExternalInput")
    out = nc.dram_tensor("out", [RS, COLS], f32, kind="ExternalOutput")

    with (
        nc.sbuf_tensor("stage", [1, 128], i32) as stage,
        nc.Block(no_gpsimd_drain=True) as block,
        nc.semaphore("ssem") as ssem,
        nc.semaphore("vsem") as vsem,
        nc.semaphore("bsem") as bsem,
        nc.semaphore("asem") as asem,
    ):

        @block.sync
        def _(sync):
            sync.dma_start(stage[0:1, 0:64], meta[0:1, 0:64]).then_inc(ssem, 16)

        @block.vector
        def _(vector):
            vector.wait_ge(ssem, 16)
            st = stage
            step = [0]

            def chain(inst):
                step[0] += 1
                inst.then_inc(vsem, 1)
                vector.wait_ge(vsem, step[0])

            # eq[64:88] = (fps == probe_tiled) as int32 0/1 (bitwise equality)
            chain(
                vector.tensor_tensor(
                    st[0:1, 64:88],
                    st[0:1, 0:24],
                    st[0:1, 24:48],
                    mybir.AluOpType.is_equal,
                )
            )
            # all4[88:94] = min over each fingerprint's 4 equality bits
            eq_v = st[0:1, 64:88].rearrange("p (a b) -> p a b", a=6)
            chain(
                vector.tensor_reduce(
                    st[0:1, 88:94], eq_v, mybir.AxisListType.X, mybir.AluOpType.min
                )
            )
            # score[94:100] = all4 * [8,7,6,5,4,3] (weights staged at [48:54])
            chain(
                vector.tensor_tensor(
                    st[0:1, 94:100],
                    st[0:1, 88:94],
                    st[0:1, 48:54],
                    mybir.AluOpType.mult,
                )
            )
            # m[100:101] = max(score) = 8 - first_match (0 if no match).
            # The issuing engines turn m into the index with register ALU
            # ops, which are cheaper than two more DVE ops here.
            chain(
                vector.tensor_reduce(
                    st[0:1, 100:101],
                    st[0:1, 94:100],
                    mybir.AxisListType.X,
                    mybir.AluOpType.max,
                )
            )

        def interleaved(ap, groups, f):
            # [r, COLS] region traversed as [groups, m, f*COLS]: 4-row (64KB)
            # blocks are dealt round-robin to `groups` outer slots, and the
            # strided outer dim survives AP optimization, pinning the SDMA
            # engine grouping to engines 0..groups-1. Same pattern on both
            # sides of the DMA keeps the element mapping the identity.
            if len(ap.shape) == 3:  # dynamic [1, r, COLS] slice of cached
                return ap.rearrange("q (m x f) c -> (q x) m (f c)", x=groups, f=f)
            return ap.rearrange("(m x f) c -> x m (f c)", x=groups, f=f)

        def load_idx(eng, name):
            # idx = (8 - m) & 7: first-match index, no-match m=0 -> 8&7 = 0.
            with eng.register(name) as r:
                eng.reg_load(r, stage[0:1, 100:101])
                eng.reg_alu(r, 8, r, mybir.AluOpType.subtract)
                eng.reg_alu(r, r, 7, mybir.AluOpType.bitwise_and)
                return eng.snap(r, min_val=0, max_val=N_CASES - 1)

        @block.sync
        def _(sync):
            sync.wait_ge(vsem, 4)
            idx = load_idx(sync, "idxr")
            rows = slice(0, rows_a)
            sync.dma_start(
                out[rows, :], cached[bass.ds(idx, 1), rows, :]
            ).then_inc(bsem, 16)
            sync.wait_ge(bsem, 16)

        @block.scalar
        def _(scalar):
            if rows_b == 0:
                return
            scalar.wait_ge(vsem, 4)
            idx2 = load_idx(scalar, "idxa")
            rows = slice(rows_a, RS)
            f = 4 if COLS == 4096 else 1
            scalar.dma_start(
                interleaved(out[rows, :], groups_b, f),
                interleaved(cached[bass.ds(idx2, 1), rows, :], groups_b, f),
            ).then_inc(asem, 16)
            scalar.wait_ge(asem, 16)

    return nc


def build_static(rows_a=None, rows_b=None, groups_a=16, groups_b=15, idx_const=3):
    """Probe variant: same copy structure as build() but NO meta DMA / select —
    the slab index is hardcoded. Isolates the select-path cost and the
    framework pre/postamble. NOT a valid deliverable (cheats the select)."""
    rows_a = rows_a or ROWS_A
    rows_b = rows_b if rows_b is not None else ROWS_B
    nc = bass.Bass()
    f32 = mybir.dt.float32
    i32 = mybir.dt.int32

    nc.dram_tensor("meta", [2, 32], i32, kind="ExternalInput")  # unused
    cached = nc.dram_tensor("cached", [N_CASES, RS, COLS], f32, kind="ExternalInput")
    out = nc.dram_tensor("out", [RS, COLS], f32, kind="ExternalOutput")

    with (
        nc.Block(no_gpsimd_drain=True) as block,
        nc.semaphore("bsem") as bsem,
        nc.semaphore("asem") as asem,
    ):

        def interleaved(ap, groups, f):
            if len(ap.shape) == 3:
                return ap.rearrange("q (m x f) c -> (q x) m (f c)", x=groups, f=f)
            return ap.rearrange("(m x f) c -> x m (f c)", x=groups, f=f)

        @block.sync
        def _(sync):
            rows = slice(0, rows_a)
            sync.dma_start(out[rows, :], cached[idx_const, rows, :]).then_inc(
                bsem, 16
            )
            sync.wait_ge(bsem, 16)

        @block.scalar
        def _(scalar):
            if rows_b == 0:
                return
            rows = slice(rows_a, RS)
            f = 4 if COLS == 4096 else 1
            scalar.dma_start(
                interleaved(out[rows, :], groups_b, f),
                interleaved(cached[idx_const, rows, :], groups_b, f),
            ).then_inc(asem, 16)
            scalar.wait_ge(asem, 16)

    return nc


def build_v2(rows_a=None, rows_b=None, groups_a=16, groups_b=15):
    """Lean-select variant: meta staged as [2,32] (2 descriptors, no 16-way
    spray), 4 DVE ops ending in idx directly (max of hit[i]*i, no-match -> 0),
    no snap/mask on the issuing engines."""
    rows_a = rows_a or ROWS_A
    rows_b = rows_b if rows_b is not None else ROWS_B
    nc = bass.Bass()
    f32 = mybir.dt.float32
    i32 = mybir.dt.int32

    meta = nc.dram_tensor("meta", [2, 32], i32, kind="ExternalInput")
    cached = nc.dram_tensor("cached", [N_CASES, RS, COLS], f32, kind="ExternalInput")
    out = nc.dram_tensor("out", [RS, COLS], f32, kind="ExternalOutput")

    with (
        nc.sbuf_tensor("stage", [1, 128], i32) as stage,
        nc.Block(no_gpsimd_drain=True) as block,
        nc.semaphore("ssem") as ssem,
        nc.semaphore("vsem") as vsem,
        nc.semaphore("bsem") as bsem,
        nc.semaphore("asem") as asem,
    ):

        @block.sync
        def _(sync):
            sync.dma_start(stage[0:1, 0:64], meta[0:2, 0:32]).then_inc(ssem, 16)

        @block.vector
        def _(vector):
            vector.wait_ge(ssem, 16)
            st = stage
            step = [0]

            def chain(inst):
                step[0] += 1
                inst.then_inc(vsem, 1)
                vector.wait_ge(vsem, step[0])

            # eq[p0 32:56] = (fps == probe_tiled): p0[0:24] vs p1[0:24]
            chain(
                vector.tensor_tensor(
                    st[0:1, 64:88],
                    st[0:1, 0:24],
                    st[0:1, 24:48],
                    mybir.AluOpType.is_equal,
                )
            )
            # hit[p0 56:62] = min over each fingerprint's 4 equality bits
            eq_v = st[0:1, 64:88].rearrange("p (a b) -> p a b", a=6)
            chain(
                vector.tensor_reduce(
                    st[0:1, 88:94], eq_v, mybir.AxisListType.X, mybir.AluOpType.min
                )
            )
            # score[p0 62:68] = hit * [0,1,2,3,4,5] (weights staged at p0 24:30)
            chain(
                vector.tensor_tensor(
                    st[0:1, 94:100],
                    st[0:1, 88:94],
                    st[0:1, 48:54],
                    mybir.AluOpType.mult,
                )
            )
            # idx[p0 68] = max(score): the matching case index, 0 if no match.
            chain(
                vector.tensor_reduce(
                    st[0:1, 100:101],
                    st[0:1, 94:100],
                    mybir.AxisListType.X,
                    mybir.AluOpType.max,
                )
            )

        def interleaved(ap, groups, f):
            if len(ap.shape) == 3:
                return ap.rearrange("q (m x f) c -> (q x) m (f c)", x=groups, f=f)
            return ap.rearrange("(m x f) c -> x m (f c)", x=groups, f=f)

        def load_idx(eng, name):
            with eng.register(name) as r:
                eng.reg_load(r, stage[0:1, 100:101])
                return eng.snap(r, donate=True, min_val=0, max_val=N_CASES - 1)

        @block.sync
        def _(sync):
            sync.wait_ge(vsem, 4)
            idx = load_idx(sync, "idxr")
            rows = slice(0, rows_a)
            sync.dma_start(
                out[rows, :], cached[bass.ds(idx, 1), rows, :]
            ).then_inc(bsem, 16)
            sync.wait_ge(bsem, 16)

        @block.scalar
        def _(scalar):
            if rows_b == 0:
                return
            scalar.wait_ge(vsem, 4)
            idx2 = load_idx(scalar, "idxa")
            rows = slice(rows_a, RS)
            f = 4 if COLS == 4096 else 1
            scalar.dma_start(
                interleaved(out[rows, :], groups_b, f),
                interleaved(cached[bass.ds(idx2, 1), rows, :], groups_b, f),
            ).then_inc(asem, 16)
            scalar.wait_ge(asem, 16)

    return nc


def make_meta_v2(probe, fps):
    flat = np.zeros(64, dtype=np.int32)
    flat[0:24] = fps.reshape(-1).view(np.int32)
    flat[24:48] = np.tile(probe.reshape(-1), 6).view(np.int32)
    flat[48:54] = np.arange(6, dtype=np.int32)
    return flat.reshape(2, 32)


def build_v3(rows_a=None, rows_b=None, groups_b=15):
    """v2's lean select, but both copy parts issued from the SP engine on the
    single qSPDynamicHW ring (part B still engine-15-hedged via the
    15-group interleave). Tests whether NRT's dma_rearm postamble scales
    with the number of HWDGE rings used."""
    rows_a = rows_a or ROWS_A
    rows_b = rows_b if rows_b is not None else ROWS_B
    nc = bass.Bass()
    f32 = mybir.dt.float32
    i32 = mybir.dt.int32

    meta = nc.dram_tensor("meta", [2, 32], i32, kind="ExternalInput")
    cached = nc.dram_tensor("cached", [N_CASES, RS, COLS], f32, kind="ExternalInput")
    out = nc.dram_tensor("out", [RS, COLS], f32, kind="ExternalOutput")

    with (
        nc.sbuf_tensor("stage", [1, 128], i32) as stage,
        nc.Block(no_gpsimd_drain=True) as block,
        nc.semaphore("ssem") as ssem,
        nc.semaphore("vsem") as vsem,
        nc.semaphore("bsem") as bsem,
    ):

        @block.sync
        def _(sync):
            sync.dma_start(stage[0:1, 0:64], meta[0:2, 0:32]).then_inc(ssem, 16)

        @block.vector
        def _(vector):
            vector.wait_ge(ssem, 16)
            st = stage
            step = [0]

            def chain(inst):
                step[0] += 1
                inst.then_inc(vsem, 1)
                vector.wait_ge(vsem, step[0])

            chain(
                vector.tensor_tensor(
                    st[0:1, 64:88],
                    st[0:1, 0:24],
                    st[0:1, 24:48],
                    mybir.AluOpType.is_equal,
                )
            )
            eq_v = st[0:1, 64:88].rearrange("p (a b) -> p a b", a=6)
            chain(
                vector.tensor_reduce(
                    st[0:1, 88:94], eq_v, mybir.AxisListType.X, mybir.AluOpType.min
                )
            )
            chain(
                vector.tensor_tensor(
                    st[0:1, 94:100],
                    st[0:1, 88:94],
                    st[0:1, 48:54],
                    mybir.AluOpType.mult,
                )
            )
            chain(
                vector.tensor_reduce(
                    st[0:1, 100:101],
                    st[0:1, 94:100],
                    mybir.AxisListType.X,
                    mybir.AluOpType.max,
                )
            )

        def interleaved(ap, groups, f):
            if len(ap.shape) == 3:
                return ap.rearrange("q (m x f) c -> (q x) m (f c)", x=groups, f=f)
            return ap.rearrange("(m x f) c -> x m (f c)", x=groups, f=f)

        @block.sync
        def _(sync):
            sync.wait_ge(vsem, 4)
            with sync.register("idxr") as r:
                sync.reg_load(r, stage[0:1, 100:101])
                idx = sync.snap(r, donate=True, min_val=0, max_val=N_CASES - 1)
            rows = slice(0, rows_a)
            sync.dma_start(
                out[rows, :], cached[bass.ds(idx, 1), rows, :]
            ).then_inc(bsem, 16)
            rows = slice(rows_a, RS)
            f = 4 if COLS == 4096 else 1
            sync.dma_start(
                interleaved(out[rows, :], groups_b, f),
                interleaved(cached[bass.ds(idx, 1), rows, :], groups_b, f),
            ).then_inc(bsem, 16)
            sync.wait_ge(bsem, 32)

    return nc


def build_v4(rows_a=None, rows_b=None, groups_b=15):
    """v2's lean select without the nc.Block wrapper: instructions are
    emitted straight into the main body, so there are no per-engine block
    branches, no exit drains, and no bass exit barrier (NRT's postamble
    barrier already serializes program end)."""
    rows_a = rows_a or ROWS_A
    rows_b = rows_b if rows_b is not None else ROWS_B
    nc = bass.Bass()
    f32 = mybir.dt.float32
    i32 = mybir.dt.int32

    meta = nc.dram_tensor("meta", [2, 32], i32, kind="ExternalInput")
    cached = nc.dram_tensor("cached", [N_CASES, RS, COLS], f32, kind="ExternalInput")
    out = nc.dram_tensor("out", [RS, COLS], f32, kind="ExternalOutput")

    stage_cm = nc.sbuf_tensor("stage", [1, 128], i32)
    stage = stage_cm.__enter__()
    ssem_cm = nc.semaphore("ssem")
    ssem = ssem_cm.__enter__()
    vsem_cm = nc.semaphore("vsem")
    vsem = vsem_cm.__enter__()
    bsem_cm = nc.semaphore("bsem")
    bsem = bsem_cm.__enter__()
    asem_cm = nc.semaphore("asem")
    asem = asem_cm.__enter__()

    sync, vector, scalar = nc.sync, nc.vector, nc.scalar

    sync.dma_start(stage[0:1, 0:64], meta[0:2, 0:32]).then_inc(ssem, 16)

    vector.wait_ge(ssem, 16)
    st = stage
    step = [0]

    def chain(inst):
        step[0] += 1
        inst.then_inc(vsem, 1)
        vector.wait_ge(vsem, step[0])

    chain(
        vector.tensor_tensor(
            st[0:1, 64:88], st[0:1, 0:24], st[0:1, 24:48], mybir.AluOpType.is_equal
        )
    )
    eq_v = st[0:1, 64:88].rearrange("p (a b) -> p a b", a=6)
    chain(
        vector.tensor_reduce(
            st[0:1, 88:94], eq_v, mybir.AxisListType.X, mybir.AluOpType.min
        )
    )
    chain(
        vector.tensor_tensor(
            st[0:1, 94:100], st[0:1, 88:94], st[0:1, 48:54], mybir.AluOpType.mult
        )
    )
    chain(
        vector.tensor_reduce(
            st[0:1, 100:101], st[0:1, 94:100], mybir.AxisListType.X,
            mybir.AluOpType.max,
        )
    )

    def interleaved(ap, groups, f):
        if len(ap.shape) == 3:
            return ap.rearrange("q (m x f) c -> (q x) m (f c)", x=groups, f=f)
        return ap.rearrange("(m x f) c -> x m (f c)", x=groups, f=f)

    def load_idx(eng, name):
        with eng.register(name) as r:
            eng.reg_load(r, stage[0:1, 100:101])
            return eng.snap(r, donate=True, min_val=0, max_val=N_CASES - 1)

    sync.wait_ge(vsem, 4)
    idx = load_idx(sync, "idxr")
    rows = slice(0, rows_a)
    sync.dma_start(out[rows, :], cached[bass.ds(idx, 1), rows, :]).then_inc(bsem, 16)

    scalar.wait_ge(vsem, 4)
    idx2 = load_idx(scalar, "idxa")
    rows = slice(rows_a, RS)
    f = 4 if COLS == 4096 else 1
    scalar.dma_start(
        interleaved(out[rows, :], groups_b, f),
        interleaved(cached[bass.ds(idx2, 1), rows, :], groups_b, f),
    ).then_inc(asem, 16)

    sync.wait_ge(bsem, 16)
    scalar.wait_ge(asem, 16)

    return nc


ROWS_A0 = 64  # starter: 1 descriptor per engine, doorbell ~0.5us earlier


def build_v5(lean_init=False):
    """v4 + split part A into a 16-descriptor starter DMA (rows 0:64) and
    the rest, so the SDMA doorbell lands ~0.5us earlier. With lean_init,
    additionally suppress the framework's const-AP memsets and init
    all-engine barrier, which sit inside the measured window."""
    import contextlib

    @contextlib.contextmanager
    def _lean():
        if not lean_init:
            yield
            return
        orig_barrier = bass.Bass.all_engine_barrier
        orig_memset = bass.BassGpSimd.memset
        bass.Bass.all_engine_barrier = lambda self, *a, **k: None
        bass.BassGpSimd.memset = lambda self, ap, c: None
        try:
            yield
        finally:
            bass.Bass.all_engine_barrier = orig_barrier
            bass.BassGpSimd.memset = orig_memset

    rows_a = ROWS_A
    rows_b = ROWS_B
    groups_b = 15
    with _lean():
        nc = bass.Bass()
    f32 = mybir.dt.float32
    i32 = mybir.dt.int32

    meta = nc.dram_tensor("meta", [2, 32], i32, kind="ExternalInput")
    cached = nc.dram_tensor("cached", [N_CASES, RS, COLS], f32, kind="ExternalInput")
    out = nc.dram_tensor("out", [RS, COLS], f32, kind="ExternalOutput")

    stage = nc.sbuf_tensor("stage", [1, 128], i32).__enter__()
    ssem = nc.semaphore("ssem").__enter__()
    vsem = nc.semaphore("vsem").__enter__()
    bsem = nc.semaphore("bsem").__enter__()
    asem = nc.semaphore("asem").__enter__()

    sync, vector, scalar = nc.sync, nc.vector, nc.scalar

    sync.dma_start(stage[0:1, 0:64], meta[0:2, 0:32]).then_inc(ssem, 16)

    vector.wait_ge(ssem, 16)
    st = stage
    step = [0]

    def chain(inst):
        step[0] += 1
        inst.then_inc(vsem, 1)
        vector.wait_ge(vsem, step[0])

    chain(
        vector.tensor_tensor(
            st[0:1, 64:88], st[0:1, 0:24], st[0:1, 24:48], mybir.AluOpType.is_equal
        )
    )
    eq_v = st[0:1, 64:88].rearrange("p (a b) -> p a b", a=6)
    chain(
        vector.tensor_reduce(
            st[0:1, 88:94], eq_v, mybir.AxisListType.X, mybir.AluOpType.min
        )
    )
    chain(
        vector.tensor_tensor(
            st[0:1, 94:100], st[0:1, 88:94], st[0:1, 48:54], mybir.AluOpType.mult
        )
    )
    chain(
        vector.tensor_reduce(
            st[0:1, 100:101], st[0:1, 94:100], mybir.AxisListType.X,
            mybir.AluOpType.max,
        )
    )

    def interleaved(ap, groups, f):
        if len(ap.shape) == 3:
            return ap.rearrange("q (m x f) c -> (q x) m (f c)", x=groups, f=f)
        return ap.rearrange("(m x f) c -> x m (f c)", x=groups, f=f)

    def load_idx(eng, name):
        with eng.register(name) as r:
            eng.reg_load(r, stage[0:1, 100:101])
            return eng.snap(r, donate=True, min_val=0, max_val=N_CASES - 1)

    sync.wait_ge(vsem, 4)
    idx = load_idx(sync, "idxr")
    r0 = slice(0, ROWS_A0)
    sync.dma_start(out[r0, :], cached[bass.ds(idx, 1), r0, :]).then_inc(bsem, 16)
    r1 = slice(ROWS_A0, rows_a)
    sync.dma_start(out[r1, :], cached[bass.ds(idx, 1), r1, :]).then_inc(bsem, 16)

    scalar.wait_ge(vsem, 4)
    idx2 = load_idx(scalar, "idxa")
    rows = slice(rows_a, RS)
    f = 4 if COLS == 4096 else 1
    scalar.dma_start(
        interleaved(out[rows, :], groups_b, f),
        interleaved(cached[bass.ds(idx2, 1), rows, :], groups_b, f),
    ).then_inc(asem, 16)

    sync.wait_ge(bsem, 32)
    scalar.wait_ge(asem, 16)

    return nc


def build_v6():
    return build_v5(lean_init=True)


VARIANTS = {
    "base": (None, None),  # filled below
    "static": (build_static, "make_meta_v2"),
    "v2": (build_v2, "make_meta_v2"),
    "v3": (build_v3, "make_meta_v2"),
    "v4": (build_v4, "make_meta_v2"),
    "v5": (build_v5, "make_meta_v2"),
    "v6": (build_v6, "make_meta_v2"),
}


def build_variant(name):
    if name == "base":
        return build()
    return VARIANTS[name][0]()


def make_meta_variant(name, probe, fps):
    if name == "base":
        return make_meta(probe, fps)
    return make_meta_v2(probe, fps)


def make_meta(probe, fps):
    buf = np.zeros((1, 64), dtype=np.int32)
    buf[0, 0:24] = fps.reshape(-1).view(np.int32)
    buf[0, 24:48] = np.tile(probe.reshape(-1), 6).view(np.int32)
    buf[0, 48:54] = np.array([8, 7, 6, 5, 4, 3], dtype=np.int32)
    return buf


def run(inputs, trace=False, **spmd_kwargs):
    x = np.asarray(inputs["x"], dtype=np.float32)
    fingerprints = np.asarray(inputs["fingerprints"], dtype=np.float32)
    cached_outputs = np.asarray(inputs["cached_outputs"], dtype=np.float32)

    nc = build()
    meta = make_meta(x.reshape(-1)[:4], fingerprints)
    in_maps = []
    for c in range(N_CORES):
        shard = np.ascontiguousarray(cached_outputs[:, c * RS : (c + 1) * RS, :])
        in_maps.append({"meta": meta, "cached": shard})

    res = run_bass_kernel_spmd(
        nc, in_maps, list(range(N_CORES)), trace=trace, **spmd_kwargs
    )
    out = np.concatenate([res.results[c]["out"] for c in range(N_CORES)], axis=0)
    return out.astype(np.float32), res


def kernel(**inputs) -> np.ndarray:
    out, _ = run(inputs, trace=False)
    return out



# revision 8
# speedup vs baseline: 1.0493x; 1.0015x over previous
"""Content-addressed cache-select kernel for Trainium2 (8 NeuronCores, SPMD).

Problem: out = cached_outputs[idx] where idx is the first row of
`fingerprints` (6x4) exactly equal to the first 4 floats of `x`, else 0.

Strategy (row-parallel over 8 cores):
  - Each core receives its 2048-row shard of all 6 cached slabs plus a
    small staged "meta" block (fingerprints, the replicated probe tiled
    x6, and index weights 0..5) packed on the host.
  - On device: one DMA stages meta into SBUF; the vector engine computes
    idx in 4 small int32 ops (bitwise equality == float equality for
    these inputs): eq = (fps == probe), hit = min-reduce per case,
    score = hit * [0..5], idx = max(score) (0 when no match, matching
    the reference's argmax-of-all-False). The SP and ACT engines
    reg-load idx and issue dynamic-offset DRAM->DRAM DMAs copying the
    selected 32MB slab shard to the output.
  - The copy is bound by the per-NC HBM path (~21 GB/s per SDMA engine
    when all 16 run, ~690 GB/s r+w aggregate): ~102-104us for
    32 MiB read + 32 MiB write. Descriptor split: part A (contiguous,
    SP queue) gives every engine 17 64KB descriptors - with a 16-desc
    starter DMA first so the SDMA doorbell lands ~0.5us early; part B
    (4-row-block interleaved AP, ACT queue) gives engines 0-14 another
    16, hedging engine 15 at half load (free under the HBM ceiling).
  - Raw engine streams without nc.Block: no per-engine block branches,
    exit drains, or bass exit barrier (NRT's injected postamble barrier
    already serializes program end). The framework's const-AP memsets
    and init all-engine barrier are suppressed during Bass construction;
    they emit dead instructions ahead of the kernel body (the init
    gpsimd sem_clear is still ordered by the NRT pseudo sync barrier).
"""
import contextlib

import numpy as np

import concourse.bass as bass
import concourse.mybir as mybir
from concourse.bass_utils import run_bass_kernel_spmd

N_CASES = 6
ROWS, COLS = 16384, 4096
N_CORES = 8
RS = ROWS // N_CORES  # rows per core

# Part A (SP queue): 1088 rows = 16 groups x 68 rows -> 17 x 64KB
# descriptors per engine, issued as a 64-row starter (1 desc/engine)
# plus the remaining 1024 rows. Part B (ACT queue): 960 rows = 15
# groups x 64 rows -> 16 x 64KB per engine on engines 0-14 only.
ROWS_A = 1088
ROWS_A0 = 64
ROWS_B = RS - ROWS_A  # 960
GROUPS_B = 15


@contextlib.contextmanager
def _lean_bass_init():
    """Suppress the framework's const-AP memsets and init all-engine
    barrier while constructing Bass. Neither is needed here: the const
    APs have no readers in this program, and the NRT pseudo sync
    barrier emitted earlier in init already orders the gpsimd semaphore
    clear against every engine's kernel body."""
    orig_barrier = bass.Bass.all_engine_barrier
    orig_memset = bass.BassGpSimd.memset
    bass.Bass.all_engine_barrier = lambda self, *a, **k: None
    bass.BassGpSimd.memset = lambda self, ap, c: None
    try:
        yield
    finally:
        bass.Bass.all_engine_barrier = orig_barrier
        bass.BassGpSimd.memset = orig_memset


def build():
    with _lean_bass_init():
        nc = bass.Bass()
    f32 = mybir.dt.float32
    i32 = mybir.dt.int32

    meta = nc.dram_tensor("meta", [2, 32], i32, kind="ExternalInput")
    cached = nc.dram_tensor("cached", [N_CASES, RS, COLS], f32, kind="ExternalInput")
    out = nc.dram_tensor("out", [RS, COLS], f32, kind="ExternalOutput")

    stage = nc.sbuf_tensor("stage", [1, 128], i32).__enter__()
    ssem = nc.semaphore("ssem").__enter__()
    vsem = nc.semaphore("vsem").__enter__()
    bsem = nc.semaphore("bsem").__enter__()
    asem = nc.semaphore("asem").__enter__()

    sync, vector, scalar = nc.sync, nc.vector, nc.scalar
    st = stage

    # Stage meta into SBUF partition 0 (DVE operands must not carry a
    # partition offset, so everything lives on one partition).
    sync.dma_start(st[0:1, 0:64], meta[0:2, 0:32]).then_inc(ssem, 16)

    vector.wait_ge(ssem, 16)
    step = [0]

    def chain(inst):
        # Same-engine RAW hazard fence: DVE is pipelined, so each op
        # waits for the previous one's semaphore before reading its
        # output.
        step[0] += 1
        inst.then_inc(vsem, 1)
        vector.wait_ge(vsem, step[0])

    # eq[64:88] = (fps == probe_tiled) as int32 0/1
    chain(
        vector.tensor_tensor(
            st[0:1, 64:88], st[0:1, 0:24], st[0:1, 24:48], mybir.AluOpType.is_equal
        )
    )
    # hit[88:94] = min over each fingerprint's 4 equality bits
    eq_v = st[0:1, 64:88].rearrange("p (a b) -> p a b", a=6)
    chain(
        vector.tensor_reduce(
            st[0:1, 88:94], eq_v, mybir.AxisListType.X, mybir.AluOpType.min
        )
    )
    # score[94:100] = hit * [0,1,2,3,4,5] (weights staged at [48:54])
    chain(
        vector.tensor_tensor(
            st[0:1, 94:100], st[0:1, 88:94], st[0:1, 48:54], mybir.AluOpType.mult
        )
    )
    # idx[100] = max(score): the matching case index, 0 if no match.
    chain(
        vector.tensor_reduce(
            st[0:1, 100:101],
            st[0:1, 94:100],
            mybir.AxisListType.X,
            mybir.AluOpType.max,
        )
    )

    def interleaved(ap, groups, f):
        # [r, COLS] region traversed as [groups, m, f*COLS]: 4-row (64KB)
        # blocks are dealt round-robin to `groups` outer slots, and the
        # strided outer dim survives AP optimization, pinning the SDMA
        # engine grouping to engines 0..groups-1. Same pattern on both
        # sides of the DMA keeps the element mapping the identity.
        if len(ap.shape) == 3:  # dynamic [1, r, COLS] slice of cached
            return ap.rearrange("q (m x f) c -> (q x) m (f c)", x=groups, f=f)
        return ap.rearrange("(m x f) c -> x m (f c)", x=groups, f=f)

    def load_idx(eng, name):
        with eng.register(name) as r:
            eng.reg_load(r, st[0:1, 100:101])
            return eng.snap(r, donate=True, min_val=0, max_val=N_CASES - 1)

    sync.wait_ge(vsem, 4)
    idx = load_idx(sync, "idxr")
    r0 = slice(0, ROWS_A0)
    sync.dma_start(out[r0, :], cached[bass.ds(idx, 1), r0, :]).then_inc(bsem, 16)
    r1 = slice(ROWS_A0, ROWS_A)
    sync.dma_start(out[r1, :], cached[bass.ds(idx, 1), r1, :]).then_inc(bsem, 16)

    scalar.wait_ge(vsem, 4)
    idx2 = load_idx(scalar, "idxa")
    rows = slice(ROWS_A, RS)
    f = 4 if COLS == 4096 else 1
    scalar.dma_start(
        interleaved(out[rows, :], GROUPS_B, f),
        interleaved(cached[bass.ds(idx2, 1), rows, :], GROUPS_B, f),
    ).then_inc(asem, 16)

    sync.wait_ge(bsem, 32)
    scalar.wait_ge(asem, 16)

    return nc


def make_meta(probe, fps):
    flat = np.zeros(64, dtype=np.int32)
    flat[0:24] = fps.reshape(-1).view(np.int32)
    flat[24:48] = np.tile(probe.reshape(-1), 6).view(np.int32)
    flat[48:54] = np.arange(6, dtype=np.int32)
    return flat.reshape(2, 32)


def run(inputs, trace=False, **spmd_kwargs):
    x = np.asarray(inputs["x"], dtype=np.float32)
    fingerprints = np.asarray(inputs["fingerprints"], dtype=np.float32)
    cached_outputs = np.asarray(inputs["cached_outputs"], dtype=np.float32)

    nc = build()
    meta = make_meta(x.reshape(-1)[:4], fingerprints)
    in_maps = []
    for c in range(N_CORES):
        shard = np.ascontiguousarray(cached_outputs[:, c * RS : (c + 1) * RS, :])
        in_maps.append({"meta": meta, "cached": shard})

    res = run_bass_kernel_spmd(
        nc, in_maps, list(range(N_CORES)), trace=trace, **spmd_kwargs
    )
    out = np.concatenate([res.results[c]["out"] for c in range(N_CORES)], axis=0)
    return out.astype(np.float32), res


def kernel(**inputs) -> np.ndarray:
    out, _ = run(inputs, trace=False)
    return out


# revision 9
# speedup vs baseline: 1.0686x; 1.0184x over previous
"""Content-addressed cache-select kernel for Trainium2 (8 NeuronCores, SPMD).

Problem: out = cached_outputs[idx] where idx is the first row of
`fingerprints` (6x4) exactly equal to the first 4 floats of `x`, else 0.

Strategy (row-parallel over 8 cores):
  - Each core receives its 2048-row shard of all 6 cached slabs plus a
    small staged "meta" block (fingerprints, the replicated probe tiled
    x6, and index weights 0..5) packed on the host.
  - On device: one DMA stages meta into SBUF; the vector engine computes
    idx in 4 small int32 ops (bitwise equality == float equality for
    these inputs): eq = (fps == probe), hit = min-reduce per case,
    score = hit * [0..5], idx = max(score) (0 when no match, matching
    the reference's argmax-of-all-False). The SP and ACT engines
    reg-load idx and issue dynamic-offset DRAM->DRAM DMAs copying the
    selected 32MB slab shard to the output.
  - The copy is bound by the per-NC HBM path (~21 GB/s per SDMA engine
    when all 16 run, ~690 GB/s r+w aggregate): ~102-104us for
    32 MiB read + 32 MiB write. Descriptor split: part A (contiguous,
    SP queue) gives every engine 17 64KB descriptors - with a 16-desc
    starter DMA first so the SDMA doorbell lands ~0.5us early; part B
    (4-row-block interleaved AP, ACT queue) gives engines 0-14 another
    16, hedging engine 15 at half load (free under the HBM ceiling).
  - Raw engine streams without nc.Block: no per-engine block branches,
    exit drains, or bass exit barrier (NRT's injected postamble barrier
    already serializes program end). The framework's const-AP memsets
    and init all-engine barrier are suppressed during Bass construction;
    they emit dead instructions ahead of the kernel body (the init
    gpsimd sem_clear is still ordered by the NRT pseudo sync barrier).
"""
import contextlib

import numpy as np

import concourse.bass as bass
import concourse.mybir as mybir
from concourse.bass_utils import run_bass_kernel_spmd

N_CASES = 6
ROWS, COLS = 16384, 4096
N_CORES = 8
RS = ROWS // N_CORES  # rows per core

# Part A (SP queue): 1088 rows = 16 groups x 68 rows -> 17 x 64KB
# descriptors per engine, issued as a 64-row starter (1 desc/engine)
# plus the remaining 1024 rows. Part B (ACT queue): 960 rows dealt as
# 4-row blocks round-robin to all 16 engines -> 15 x 64KB each, so
# every engine carries 32 descriptors. (Profiled runs never reproduced
# the prior session's intermittently-slow engine 15; the balanced
# split takes one descriptor off the stably-slower engines 0-7, which
# otherwise define the makespan.)
ROWS_A = 1088
ROWS_A0 = 64
ROWS_B = RS - ROWS_A  # 960
GROUPS_B = 16


@contextlib.contextmanager
def _lean_bass_init():
    """Suppress the framework's const-AP memsets and init all-engine
    barrier while constructing Bass. Neither is needed here: the const
    APs have no readers in this program, and the NRT pseudo sync
    barrier emitted earlier in init already orders the gpsimd semaphore
    clear against every engine's kernel body."""
    orig_barrier = bass.Bass.all_engine_barrier
    orig_memset = bass.BassGpSimd.memset
    bass.Bass.all_engine_barrier = lambda self, *a, **k: None
    bass.BassGpSimd.memset = lambda self, ap, c: None
    try:
        yield
    finally:
        bass.Bass.all_engine_barrier = orig_barrier
        bass.BassGpSimd.memset = orig_memset


def build():
    with _lean_bass_init():
        nc = bass.Bass()
    f32 = mybir.dt.float32
    i32 = mybir.dt.int32

    meta = nc.dram_tensor("meta", [2, 32], i32, kind="ExternalInput")
    cached = nc.dram_tensor("cached", [N_CASES, RS, COLS], f32, kind="ExternalInput")
    out = nc.dram_tensor("out", [RS, COLS], f32, kind="ExternalOutput")

    stage = nc.sbuf_tensor("stage", [1, 128], i32).__enter__()
    ssem = nc.semaphore("ssem").__enter__()
    vsem = nc.semaphore("vsem").__enter__()
    bsem = nc.semaphore("bsem").__enter__()
    asem = nc.semaphore("asem").__enter__()

    sync, vector, scalar = nc.sync, nc.vector, nc.scalar
    st = stage

    # Stage meta into SBUF partition 0 (DVE operands must not carry a
    # partition offset, so everything lives on one partition).
    sync.dma_start(st[0:1, 0:64], meta[0:2, 0:32]).then_inc(ssem, 16)

    vector.wait_ge(ssem, 16)
    step = [0]

    def chain(inst):
        # Same-engine RAW hazard fence: DVE is pipelined, so each op
        # waits for the previous one's semaphore before reading its
        # output.
        step[0] += 1
        inst.then_inc(vsem, 1)
        vector.wait_ge(vsem, step[0])

    # eq[64:88] = (fps == probe_tiled) as int32 0/1
    chain(
        vector.tensor_tensor(
            st[0:1, 64:88], st[0:1, 0:24], st[0:1, 24:48], mybir.AluOpType.is_equal
        )
    )
    # hit[88:94] = min over each fingerprint's 4 equality bits
    eq_v = st[0:1, 64:88].rearrange("p (a b) -> p a b", a=6)
    chain(
        vector.tensor_reduce(
            st[0:1, 88:94], eq_v, mybir.AxisListType.X, mybir.AluOpType.min
        )
    )
    # score[94:100] = hit * [0,1,2,3,4,5] (weights staged at [48:54])
    chain(
        vector.tensor_tensor(
            st[0:1, 94:100], st[0:1, 88:94], st[0:1, 48:54], mybir.AluOpType.mult
        )
    )
    # idx[100] = max(score): the matching case index, 0 if no match.
    chain(
        vector.tensor_reduce(
            st[0:1, 100:101],
            st[0:1, 94:100],
            mybir.AxisListType.X,
            mybir.AluOpType.max,
        )
    )

    def interleaved(ap, groups, f):
        # [r, COLS] region traversed as [groups, m, f*COLS]: 4-row (64KB)
        # blocks are dealt round-robin to `groups` outer slots, and the
        # strided outer dim survives AP optimization, pinning the SDMA
        # engine grouping to engines 0..groups-1. Same pattern on both
        # sides of the DMA keeps the element mapping the identity.
        if len(ap.shape) == 3:  # dynamic [1, r, COLS] slice of cached
            return ap.rearrange("q (m x f) c -> (q x) m (f c)", x=groups, f=f)
        return ap.rearrange("(m x f) c -> x m (f c)", x=groups, f=f)

    def load_idx(eng, name):
        with eng.register(name) as r:
            eng.reg_load(r, st[0:1, 100:101])
            return eng.snap(r, donate=True, min_val=0, max_val=N_CASES - 1)

    sync.wait_ge(vsem, 4)
    idx = load_idx(sync, "idxr")
    r0 = slice(0, ROWS_A0)
    sync.dma_start(out[r0, :], cached[bass.ds(idx, 1), r0, :]).then_inc(bsem, 16)
    r1 = slice(ROWS_A0, ROWS_A)
    sync.dma_start(out[r1, :], cached[bass.ds(idx, 1), r1, :]).then_inc(bsem, 16)

    scalar.wait_ge(vsem, 4)
    idx2 = load_idx(scalar, "idxa")
    rows = slice(ROWS_A, RS)
    f = 4 if COLS == 4096 else 1
    scalar.dma_start(
        interleaved(out[rows, :], GROUPS_B, f),
        interleaved(cached[bass.ds(idx2, 1), rows, :], GROUPS_B, f),
    ).then_inc(asem, 16)

    sync.wait_ge(bsem, 32)
    scalar.wait_ge(asem, 16)

    return nc


def make_meta(probe, fps):
    flat = np.zeros(64, dtype=np.int32)
    flat[0:24] = fps.reshape(-1).view(np.int32)
    flat[24:48] = np.tile(probe.reshape(-1), 6).view(np.int32)
    flat[48:54] = np.arange(6, dtype=np.int32)
    return flat.reshape(2, 32)


def run(inputs, trace=False, **spmd_kwargs):
    x = np.asarray(inputs["x"], dtype=np.float32)
    fingerprints = np.asarray(inputs["fingerprints"], dtype=np.float32)
    cached_outputs = np.asarray(inputs["cached_outputs"], dtype=np.float32)

    nc = build()
    meta = make_meta(x.reshape(-1)[:4], fingerprints)
    in_maps = []
    for c in range(N_CORES):
        shard = np.ascontiguousarray(cached_outputs[:, c * RS : (c + 1) * RS, :])
        in_maps.append({"meta": meta, "cached": shard})

    res = run_bass_kernel_spmd(
        nc, in_maps, list(range(N_CORES)), trace=trace, **spmd_kwargs
    )
    out = np.concatenate([res.results[c]["out"] for c in range(N_CORES)], axis=0)
    return out.astype(np.float32), res


def kernel(**inputs) -> np.ndarray:
    out, _ = run(inputs, trace=False)
    return out
